# revision 18
# baseline (speedup 1.0000x reference)
"""BiLSTM-CRF forward (log partition) on 8 trn2 NeuronCores — single fused
SPMD kernel, one launch per call.

Sharding: data-parallel over batch. Each core owns 8 of the 64 batch columns
and runs the whole pipeline for them: embedding gather (indirect DMA from a
device-resident table), input-projection GEMMs, forward+backward LSTM scans
(interleaved per step for engine overlap), encoder GEMM, emissions, and the
CRF forward scan in the exp domain with periodic rescaling. Outputs per core
are tiny (the 32 rescale factors and the final alpha vector); the host sums
logs and applies the STOP transition.

Per-call host<->device traffic is ~150KB (the axon tunnel runs ~35-60MB/s, so
the baseline's 125MB/call dominated). All weights and the embedding table are
transferred once and kept device-resident; the jitted PJRT callable is built
once and cached (a fresh closure per call costs ~9s in retrace/recompile).
Per-call wall time is bound by the tunnel's per-sync round trip (~52-125ms
depending on network phase); device execution (~1.5ms sim: scan 1.05ms,
A+B 0.2ms, CRF 0.17ms, encoder 0.07ms) pipelines entirely inside that sync
(slope-test: 9 pipelined executes cost no more wall than 1). The xp GEMMs are
interleaved into the gather loop and the CRF runs as two independent batch
halves so PE/DVE dependency chains overlap.
"""
import hashlib
import numpy as np
import ml_dtypes

import concourse.bass as bass
import concourse.mybir as mybir
import concourse.tile as tile

T, B, E, H, V, K = 512, 64, 256, 512, 50000, 50
BS = 8             # batch per core
P = 128
NG = 16            # gate tiles (4H/128)
NK = 4             # h chunks (H/128)
NE = 2             # e chunks (E/128)
GRP = 32           # scan steps per xp prefetch group
NGRP = T // GRP    # 16
TB = T * BS        # 4096
NBLK = TB // 512   # 8
NGT = TB // P      # 32 token groups of 128
CB = BS            # CRF batch per core
ZR = 16            # CRF rescale period
AF = mybir.ActivationFunctionType
BF16 = mybir.dt.bfloat16
F32 = mybir.dt.float32
I32 = mybir.dt.int32

_CACHE = {}


def _fix_sync_waits(nc, max_waits=1):
    import bass_rust
    for fn in nc.m.functions:
        for bb in fn.blocks:
            out = []
            for inst in bb.instructions:
                si = inst.sync_info
                if si is not None and si.on_wait and len(si.on_wait) > max_waits:
                    waits = list(si.on_wait)
                    extra, keep = waits[:-max_waits], waits[-max_waits:]
                    for j in range(0, len(extra), max_waits):
                        nop = mybir.InstNoOp(name=f"{inst.name}_ws{j}", ins=[], outs=[])
                        nop.engine = inst.engine
                        nop.sync_info = bass_rust.SyncInfo(
                            on_wait=extra[j:j + max_waits], on_update=[])
                        out.append(nop)
                    inst.sync_info = bass_rust.SyncInfo(
                        on_wait=keep, on_update=list(si.on_update or []))
                out.append(inst)
            bb.instructions = out


def build(upto="full"):
    # upto: "ab" | "scan" | "enc" | "full" — truncation points for sim profiling
    nc = bass.Bass()
    dp = nc.declare_dram_parameter
    tok_in = dp("tok", [NGT, P], I32, isOutput=False)          # per-call
    emb_in = dp("emb", [V, E], BF16, isOutput=False)
    perm_in = dp("perm", [P, 2, P], BF16, isOutput=False)      # ident, rev16x8
    whh_in = dp("whh", [P, 2 * NG * NK, P], BF16, isOutput=False)
    wih_in = dp("wih", [P, 2 * NG * NE, P], BF16, isOutput=False)
    bias_in = dp("bias", [P, 2, NG], F32, isOutput=False)
    wenc_in = dp("wenc", [P, 2 * NK * NK, P], BF16, isOutput=False)
    benc_in = dp("benc", [P, NK], F32, isOutput=False)
    wout_in = dp("wout", [P, NK, K], BF16, isOutput=False)
    bout_in = dp("bout", [K, 1], F32, isOutput=False)
    pp_in = dp("pp", [K, K + 1], BF16, isOutput=False)
    a0_in = dp("a0", [K, CB], BF16, isOutput=False)
    # single packed output => one d2h sync per call (each fetched jax Array
    # pays the ~72ms tunnel round trip separately)
    res_out = dp("res", [1, CB * (T // ZR) + K * CB], F32, isOutput=True)

    xp_dram = [nc.dram_tensor(f"xp{d}", [NG, P, TB], BF16) for d in range(2)]

    with tile.TileContext(nc) as tc:
        with tc.tile_pool(name="persist", bufs=1) as pers:
            whh = pers.tile([P, 2 * NG * NK, P], BF16)
            nc.sync.dma_start(whh[:], whh_in[:])
            bias = pers.tile([P, 2, NG], F32)
            nc.sync.dma_start(bias[:], bias_in[:])
            hseq = [pers.tile([P, NK, TB], BF16, name=f"hseq{_d}") for _d in range(2)]

            # ---- phase A+B: gather + transpose + xp GEMMs ----
            with (
                tc.tile_pool(name="g1", bufs=1) as c1,
                tc.tile_pool(name="w1", bufs=3) as w1,
                tc.tile_pool(name="pt", bufs=2, space="PSUM") as pst,
                tc.tile_pool(name="p1", bufs=2, space="PSUM") as ps1,
            ):
                perm = c1.tile([P, 2, P], BF16)
                nc.sync.dma_start(perm[:], perm_in[:])
                tok = c1.tile([P, NGT], I32)
                nc.sync.dma_start(tok[:], tok_in[:].rearrange("a p -> p a"))
                wih = c1.tile([P, 2 * NG * NE, P], BF16)
                nc.sync.dma_start(wih[:], wih_in[:])
                xT = [c1.tile([P, NE, TB], BF16, name=f"xT{_d}") for _d in range(2)]

                def xp_gemm(d, blk):
                    for m in range(NG):
                        ps = ps1.tile([P, 512], F32, tag="xps", name="ps")
                        for e in range(NE):
                            nc.tensor.matmul(
                                ps[:], lhsT=wih[:, (d * NG + m) * NE + e, :],
                                rhs=xT[d][:, e, blk * 512:(blk + 1) * 512],
                                start=(e == 0), stop=(e == NE - 1))
                        xo = w1.tile([P, 512], BF16, tag="xpo", name="xo")
                        nc.vector.tensor_scalar_add(
                            xo[:], ps[:], bias[:, d, m:m + 1])
                        nc.sync.dma_start(
                            xp_dram[d][m, :, blk * 512:(blk + 1) * 512], xo[:])

                # interleave the xp GEMMs with the gathers: each 4-group chunk
                # completes one 512-col block of xT_f (blk g//4) and one of
                # xT_r (blk 7-g//4), so its GEMMs can run while later groups
                # are still gathering
                for g in range(NGT):
                    emb_g = w1.tile([P, E], BF16, tag="g")
                    nc.gpsimd.indirect_dma_start(
                        out=emb_g[:], out_offset=None,
                        in_=emb_in[:],
                        in_offset=bass.IndirectOffsetOnAxis(
                            ap=tok[:, g:g + 1], axis=0),
                    )
                    for e in range(NE):
                        pt = pst.tile([P, P], BF16, tag="t")
                        nc.tensor.transpose(pt[:], emb_g[:, e * P:(e + 1) * P],
                                            perm[:, 0, :])
                        nc.vector.tensor_copy(xT[0][:, e, g * P:(g + 1) * P], pt[:])
                        pr = pst.tile([P, P], BF16, tag="r")
                        nc.tensor.transpose(pr[:], emb_g[:, e * P:(e + 1) * P],
                                            perm[:, 1, :])
                        nc.vector.tensor_copy(
                            xT[1][:, e, (NGT - 1 - g) * P:(NGT - g) * P], pr[:])
                    if g % 4 == 3:
                        xp_gemm(0, g // 4)
                        xp_gemm(1, NBLK - 1 - g // 4)

            # ---- phase C: interleaved fwd/bwd LSTM scans ----
            if upto == "ab":
                return _finish(nc)
            with (
                tc.tile_pool(name="c2", bufs=1) as c2,
                tc.tile_pool(name="p2", bufs=2, space="PSUM") as ps2,
            ):
                xr = [c2.tile([P, 2, NG, GRP, BS], BF16, name=f"xr{_d}") for _d in range(2)]
                h0 = c2.tile([P, NK, BS], BF16)
                nc.any.memset(h0[:], 0.0)
                c_t = [c2.tile([P, NK * BS], F32, name=f"c_t{_d}") for _d in range(2)]
                gs = [c2.tile([P, NG * BS], F32, name=f"gs{_d}") for _d in range(2)]
                sio = [c2.tile([P, 3 * NK * BS], F32, name=f"sio{_d}") for _d in range(2)]
                tg = [c2.tile([P, NK * BS], F32, name=f"tg{_d}") for _d in range(2)]
                m1 = [c2.tile([P, NK * BS], F32, name=f"m1_{_d}") for _d in range(2)]
                m2 = [c2.tile([P, NK * BS], F32, name=f"m2_{_d}") for _d in range(2)]
                tcc = [c2.tile([P, NK * BS], F32, name=f"tcc{_d}") for _d in range(2)]
                for d in range(2):
                    nc.any.memset(c_t[d][:], 0.0)

                def prefetch(d, g):
                    if g >= NGRP:
                        return
                    for m in range(NG):
                        nc.sync.dma_start(
                            xr[d][:, g % 2, m, :, :].rearrange("p t b -> p (t b)"),
                            xp_dram[d][m, :, g * GRP * BS:(g + 1) * GRP * BS])

                for d in range(2):
                    prefetch(d, 0)
                    prefetch(d, 1)

                def seq_col(d, i):
                    # column range in hseq[d] written at scan step i
                    at = i if d == 0 else T - 1 - i
                    return slice(at * BS, (at + 1) * BS)

                for i in range(T):
                    g, tm = divmod(i, GRP)
                    gp = []
                    for d in range(2):
                        hin = (h0[:] if i == 0
                               else hseq[d][:, :, seq_col(d, i - 1)])
                        gpd = ps2.tile([P, NG * BS], F32, tag=f"g{d}")
                        for m in range(NG):
                            for k in range(NK):
                                nc.tensor.matmul(
                                    gpd[:, m * BS:(m + 1) * BS],
                                    lhsT=whh[:, (d * NG + m) * NK + k, :],
                                    rhs=hin[:, k, :],
                                    start=(k == 0), stop=(k == NK - 1))
                        gp.append(gpd)
                    for d in range(2):
                        nc.vector.tensor_tensor(
                            gs[d][:].rearrange("p (m b) -> p m b", b=BS),
                            gp[d][:].rearrange("p (m b) -> p m b", b=BS),
                            xr[d][:, g % 2, :, tm, :], mybir.AluOpType.add)
                        nc.scalar.activation(sio[d][:], gs[d][:, 0:3 * NK * BS],
                                             AF.Sigmoid)
                        nc.scalar.activation(tg[d][:], gs[d][:, 3 * NK * BS:],
                                             AF.Tanh)
                        nc.vector.tensor_mul(m1[d][:], sio[d][:, 0:NK * BS], tg[d][:])
                        nc.vector.tensor_mul(m2[d][:], sio[d][:, NK * BS:2 * NK * BS],
                                             c_t[d][:])
                        nc.vector.tensor_add(c_t[d][:], m1[d][:], m2[d][:])
                        nc.scalar.activation(tcc[d][:], c_t[d][:], AF.Tanh)
                        nc.vector.tensor_mul(
                            hseq[d][:, :, seq_col(d, i)],
                            sio[d][:, 2 * NK * BS:3 * NK * BS].rearrange(
                                "p (a b) -> p a b", b=BS),
                            tcc[d][:].rearrange("p (a b) -> p a b", b=BS))
                    if tm == GRP - 1:
                        for d in range(2):
                            prefetch(d, g + 2)

            # ---- phase D: encoder + emissions;  phase E: CRF scan ----
            if upto == "scan":
                return _finish(nc)
            with tc.tile_pool(name="c3", bufs=1) as c3:
                ps3_cm = tc.tile_pool(name="p3", bufs=2, space="PSUM")
                ps3 = ps3_cm.__enter__()
                wenc = c3.tile([P, 2 * NK * NK, P], BF16)
                nc.sync.dma_start(wenc[:], wenc_in[:])
                benc = c3.tile([P, NK], F32)
                nc.sync.dma_start(benc[:], benc_in[:])
                wout = c3.tile([P, NK, K], BF16)
                nc.sync.dma_start(wout[:], wout_in[:])
                bout = c3.tile([K, 1], F32)
                nc.sync.dma_start(bout[:], bout_in[:])
                states = c3.tile([P, NK, TB], BF16)

                for blk in range(NBLK):
                    sl = slice(blk * 512, (blk + 1) * 512)
                    for m in range(NK):
                        ps = ps3.tile([P, 512], F32, tag="enc")
                        for k in range(NK):
                            nc.tensor.matmul(ps[:], lhsT=wenc[:, m * NK + k, :],
                                             rhs=hseq[0][:, k, sl],
                                             start=(k == 0), stop=False)
                        for k in range(NK):
                            nc.tensor.matmul(ps[:], lhsT=wenc[:, 16 + m * NK + k, :],
                                             rhs=hseq[1][:, k, sl], start=False,
                                             stop=(k == NK - 1))
                        nc.scalar.activation(states[:, m, sl], ps[:], AF.Tanh,
                                             bias=benc[:, m:m + 1])

                expE = c3.tile([K, TB], F32)
                for blk in range(NBLK):
                    sl = slice(blk * 512, (blk + 1) * 512)
                    ps = ps3.tile([K, 512], F32, tag="emit")
                    for k in range(NK):
                        nc.tensor.matmul(ps[:], lhsT=wout[:, k, :],
                                         rhs=states[:, k, sl],
                                         start=(k == 0), stop=(k == NK - 1))
                    nc.scalar.activation(expE[:, sl], ps[:], AF.Exp,
                                         bias=bout[:, 0:1])

                ps3_cm.__exit__(None, None, None)
                if upto == "enc":
                    return _finish(nc)
                psE_cm = tc.tile_pool(name="pE", bufs=1, space="PSUM")
                psE = psE_cm.__enter__()
                pp = c3.tile([K, K + 1], BF16)
                nc.sync.dma_start(pp[:], pp_in[:])
                ones_r = c3.tile([1, K], BF16)
                nc.any.memset(ones_r[:], 1.0)
                A = c3.tile([K, CB], BF16)
                nc.sync.dma_start(A[:], a0_in[:])
                zbuf = c3.tile([1, CB, T // ZR], F32)
                izb = c3.tile([1, CB], F32)
                izb_bf = c3.tile([1, CB], BF16)

                # two independent batch halves: half 1's matmul runs on PE
                # while half 0's emission multiply runs on DVE, hiding the
                # PE<->DVE semaphore latency of the sequential chain
                HB = CB // 2
                for t in range(T):
                    pss = []
                    for h in range(2):
                        hs = slice(h * HB, (h + 1) * HB)
                        ps = psE.tile([K, HB], F32, tag=f"crf{h}", bufs=2,
                                      name="ps")
                        nc.tensor.matmul(ps[:], lhsT=pp[:, 0:K], rhs=A[:, hs],
                                         start=True, stop=True)
                        pss.append(ps)
                    if t % ZR == ZR - 1:
                        r = t // ZR
                        for h in range(2):
                            hs = slice(h * HB, (h + 1) * HB)
                            zps = psE.tile([1, HB], F32, tag="zps", bufs=1,
                                           name="zps")
                            nc.tensor.matmul(zps[:], lhsT=pp[:, K:K + 1],
                                             rhs=A[:, hs], start=True, stop=True)
                            nc.vector.tensor_copy(zbuf[:, hs, r], zps[:])
                            nc.vector.reciprocal(izb[:, hs], zps[:])
                            nc.vector.tensor_copy(izb_bf[:, hs], izb[:, hs])
                            zb = psE.tile([K, HB], F32, tag="zbc", bufs=1,
                                          name="zb")
                            nc.tensor.matmul(zb[:], lhsT=ones_r[:],
                                             rhs=izb_bf[:, hs],
                                             start=True, stop=True)
                            nc.vector.tensor_mul(A[:, hs], pss[h][:],
                                                 expE[:, t * CB + h * HB:
                                                      t * CB + (h + 1) * HB])
                            nc.vector.tensor_mul(A[:, hs], A[:, hs], zb[:])
                    else:
                        for h in range(2):
                            hs = slice(h * HB, (h + 1) * HB)
                            nc.vector.tensor_mul(A[:, hs], pss[h][:],
                                                 expE[:, t * CB + h * HB:
                                                      t * CB + (h + 1) * HB])

                psE_cm.__exit__(None, None, None)
                nzb = CB * (T // ZR)
                nc.sync.dma_start(
                    res_out[:, 0:nzb],
                    zbuf[:].rearrange("p a b -> p (a b)"))
                af = c3.tile([K, CB], F32)
                nc.vector.tensor_copy(af[:], A[:])
                nc.sync.dma_start(
                    res_out[0, nzb:].rearrange("(k c) -> k c", c=CB), af[:])

    return _finish(nc)


def _finish(nc):
    _fix_sync_waits(nc)
    return nc


# ---------------- cached PJRT runner ----------------

class _Runner:
    """jit-compile the SPMD callable once; keep weights device-resident.
    (The stock run_bass_kernel_spmd builds a fresh closure per call => full
    retrace + XLA recompile, ~9s/call on this runtime.)"""

    def __init__(self, nc, n_cores):
        import jax
        from jax.sharding import Mesh, PartitionSpec, NamedSharding
        from jax.experimental.shard_map import shard_map
        from concourse.bass2jax import (_bass_exec_p, partition_id_tensor,
                                        install_neuronx_cc_hook)
        install_neuronx_cc_hook()
        self.jax = jax
        self.n_cores = n_cores
        partition_name = (nc.partition_id_tensor.name
                          if nc.partition_id_tensor else None)
        in_names, out_names, out_avals, zero_outs = [], [], [], []
        for alloc in nc.m.functions[0].allocations:
            if not isinstance(alloc, mybir.MemoryLocationSet):
                continue
            name = alloc.memorylocations[0].name
            if alloc.kind == "ExternalInput":
                if name != partition_name:
                    in_names.append(name)
            elif alloc.kind == "ExternalOutput":
                out_names.append(name)
                shape = tuple(alloc.tensor_shape)
                dtype = mybir.dt.np(alloc.dtype)
                out_avals.append(jax.core.ShapedArray(shape, dtype))
                zero_outs.append(np.zeros(shape, dtype))
        self.in_names = in_names
        self.out_names = out_names
        self.out_avals = out_avals

        def _body(*args):
            operands = list(args)
            if partition_name is not None:
                operands.append(partition_id_tensor())
            outs = _bass_exec_p.bind(
                *operands,
                out_avals=tuple(out_avals),
                in_names=tuple(in_names + out_names
                               + ([partition_name] if partition_name else [])),
                out_names=tuple(out_names),
                lowering_input_output_aliases=(),
                sim_require_finite=True,
                sim_require_nnan=True,
                nc=nc,
            )
            return tuple(outs)

        devices = jax.devices()[:n_cores]
        mesh = Mesh(np.asarray(devices), ("core",))
        self.sharding = NamedSharding(mesh, PartitionSpec("core"))
        n_io = len(in_names) + len(out_names)
        self.fn = jax.jit(
            shard_map(_body, mesh=mesh,
                      in_specs=(PartitionSpec("core"),) * n_io,
                      out_specs=(PartitionSpec("core"),) * len(out_names),
                      check_rep=False),
            keep_unused=True,
        )
        self.dev_zeros = [
            jax.device_put(np.zeros((n_cores * z.shape[0], *z.shape[1:]),
                                    z.dtype), self.sharding)
            for z in zero_outs
        ]
        self.resident = {}

    def put_resident(self, name, per_core_arrays):
        cat = np.concatenate([np.asarray(a) for a in per_core_arrays], axis=0)
        self.resident[name] = self.jax.device_put(cat, self.sharding)

    def dispatch(self, **per_call):
        """Async: returns un-fetched global output arrays."""
        args = []
        for name in self.in_names:
            if name in per_call:
                v = per_call[name]
                if isinstance(v, (list, tuple)):
                    v = np.concatenate([np.asarray(a) for a in v], axis=0)
                args.append(self.jax.device_put(v, self.sharding))
            else:
                args.append(self.resident[name])
        return self.fn(*args, *self.dev_zeros)

    def fetch(self, outs):
        host = [np.asarray(o) for o in outs]
        return [
            {name: host[i].reshape(self.n_cores, *self.out_avals[i].shape)[c]
             for i, name in enumerate(self.out_names)}
            for c in range(self.n_cores)
        ]

    def __call__(self, **per_call):
        return self.fetch(self.dispatch(**per_call))


# ---------------- host-side prep ----------------

GPERM = np.concatenate([
    np.arange(0, 512), np.arange(512, 1024),
    np.arange(1536, 2048), np.arange(1024, 1536)])  # i,f,o,g tile order


def _prep_weights(w_ih_f, w_hh_f, b_f, w_ih_b, w_hh_b, b_b,
                  w_enc, b_enc, w_out, b_out, trans, embed):
    bf = ml_dtypes.bfloat16
    whh_t = np.empty((P, 2 * NG * NK, P), np.float32)
    wih_t = np.empty((P, 2 * NG * NE, P), np.float32)
    bias = np.empty((P, 2, NG), np.float32)
    for d, (wih_d, whh_d, b_d) in enumerate(
            [(w_ih_f, w_hh_f, b_f), (w_ih_b, w_hh_b, b_b)]):
        Whh = whh_d[GPERM]
        Wih = wih_d[GPERM]
        for m in range(NG):
            for k in range(NK):
                whh_t[:, (d * NG + m) * NK + k, :] = \
                    Whh[m * P:(m + 1) * P, k * P:(k + 1) * P].T
            for e in range(NE):
                wih_t[:, (d * NG + m) * NE + e, :] = \
                    Wih[m * P:(m + 1) * P, e * P:(e + 1) * P].T
        bias[:, d, :] = b_d[GPERM].reshape(NG, P).T

    wenc_t = np.empty((P, 2 * NK * NK, P), np.float32)
    for dd in range(2):
        Wd = w_enc[:, dd * H:(dd + 1) * H]
        for m in range(NK):
            for k in range(NK):
                wenc_t[:, dd * 16 + m * NK + k, :] = \
                    Wd[m * P:(m + 1) * P, k * P:(k + 1) * P].T
    benc_t = b_enc.reshape(NK, P).T.copy()
    wout_t = np.empty((P, NK, K), np.float32)
    for k in range(NK):
        wout_t[:, k, :] = w_out[:, k * P:(k + 1) * P].T

    pp = np.zeros((K, K + 1), np.float32)
    pp[:, :K] = np.exp(trans.astype(np.float64)).T.astype(np.float32)
    pp[:, K] = 1.0
    a0 = np.zeros((K, CB), np.float32)
    a0[0, :] = 1.0

    ident = np.eye(P, dtype=np.float32)
    rev = np.zeros((P, P), np.float32)
    tt, bb_ = np.meshgrid(np.arange(P // BS), np.arange(BS), indexing='ij')
    rev[(P // BS - 1 - tt) * BS + bb_, tt * BS + bb_] = 1.0
    perm = np.stack([ident, rev], axis=1)

    return {
        "emb": embed.astype(bf),
        "perm": perm.astype(bf),
        "whh": whh_t.astype(bf), "wih": wih_t.astype(bf),
        "bias": bias,
        "wenc": wenc_t.astype(bf), "benc": benc_t,
        "wout": wout_t.astype(bf), "bout": b_out.reshape(K, 1).astype(np.float32),
        "pp": pp.astype(bf), "a0": a0.astype(bf),
    }


def _weights_sig(arrs):
    h = hashlib.blake2b(digest_size=16)
    for a in arrs:
        a = np.asarray(a)
        h.update(str(a.shape).encode())
        if a.nbytes <= 1 << 20:
            h.update(np.ascontiguousarray(a).tobytes())
        else:
            h.update(np.ascontiguousarray(a.reshape(-1)[::17]).tobytes())
    return h.digest()


def _put_weights(runner, wlist, sig):
    prepped = _prep_weights(*[np.asarray(w, dtype=np.float32)
                              for w in wlist[1:]],
                            np.asarray(wlist[0], dtype=np.float32))
    for name, arr in prepped.items():
        runner.put_resident(name, [arr] * 8)
    _CACHE["sig"] = sig
    _CACHE["etstop"] = np.exp(np.asarray(wlist[-1])[K - 1].astype(np.float64))


def kernel(tokens, embed, w_ih_f, w_hh_f, b_f, w_ih_b, w_hh_b, b_b,
           w_enc, b_enc, w_out, b_out, trans):
    tokens = np.asarray(tokens)
    if tokens.dtype != np.int32:
        tokens = tokens.astype(np.int32)
    wlist = [embed, w_ih_f, w_hh_f, b_f, w_ih_b, w_hh_b, b_b,
             w_enc, b_enc, w_out, b_out, trans]

    if "runner" not in _CACHE:
        _CACHE["runner"] = _Runner(build(), 8)
        _put_weights(_CACHE["runner"], wlist, _weights_sig(wlist))
    runner = _CACHE["runner"]

    toks = [np.ascontiguousarray(tokens[:, c * BS:(c + 1) * BS])
            .reshape(NGT, P) for c in range(8)]
    # dispatch optimistically with the resident weights, then verify the
    # weight signature while the device runs; redo on mismatch (rare).
    # Retry on transient proxy/transport errors.
    last_err = None
    for attempt in range(3):
        try:
            outs = runner.dispatch(tok=toks)
            sig = _weights_sig(wlist)
            if _CACHE["sig"] != sig:
                _put_weights(runner, wlist, sig)
                outs = runner.dispatch(tok=toks)
            res = runner.fetch(outs)
            break
        except Exception as e:  # noqa: BLE001 - re-raised after retries
            last_err = e
            import time as _time
            _time.sleep(0.5 * (attempt + 1))
    else:
        raise last_err

    etstop = _CACHE["etstop"]
    nzb = CB * (T // ZR)
    out = np.empty((B,), np.float32)
    for c in range(8):
        flat = np.asarray(res[c]["res"], dtype=np.float64).reshape(-1)
        zb = flat[:nzb].reshape(CB, T // ZR)
        af = flat[nzb:].reshape(K, CB)
        lz = np.log(zb).sum(axis=1) + np.log(etstop @ af)
        out[c * BS:(c + 1) * BS] = lz.astype(np.float32)
    return out


# revision 19
# speedup vs baseline: 1.9782x; 1.9782x over previous
"""BiLSTM-CRF forward (log partition) on 8 trn2 NeuronCores — single fused
SPMD kernel, one launch per call.

Sharding: data-parallel over batch. Each core owns 8 of the 64 batch columns
and runs the whole pipeline for them: embedding gather (indirect DMA from a
device-resident table), input-projection GEMMs, forward+backward LSTM scans
(interleaved per step for engine overlap), encoder GEMM, emissions, and the
CRF forward scan in the exp domain with periodic rescaling. Outputs per core
are tiny (the 32 rescale factors and the final alpha vector); the host sums
logs and applies the STOP transition.

Per-call host<->device traffic is ~150KB (the axon tunnel runs ~35-60MB/s, so
the baseline's 125MB/call dominated). All weights and the embedding table are
transferred once and kept device-resident; the jitted PJRT callable is built
once and cached (a fresh closure per call costs ~9s in retrace/recompile).
Per-call wall time is bound by the tunnel's per-sync round trip (~52-125ms
depending on network phase); device execution (~1.5ms sim: scan 1.05ms,
A+B 0.2ms, CRF 0.17ms, encoder 0.07ms) pipelines entirely inside that sync
(slope-test: 9 pipelined executes cost no more wall than 1). The xp GEMMs are
interleaved into the gather loop and the CRF runs as two independent batch
halves so PE/DVE dependency chains overlap.
"""
import hashlib
import numpy as np
import ml_dtypes

import concourse.bass as bass
import concourse.mybir as mybir
import concourse.tile as tile

T, B, E, H, V, K = 512, 64, 256, 512, 50000, 50
BS = 8             # batch per core
P = 128
NG = 16            # gate tiles (4H/128)
NK = 4             # h chunks (H/128)
NE = 2             # e chunks (E/128)
GRP = 32           # scan steps per xp prefetch group
NGRP = T // GRP    # 16
TB = T * BS        # 4096
NBLK = TB // 512   # 8
NGT = TB // P      # 32 token groups of 128
CB = BS            # CRF batch per core
ZR = 16            # CRF rescale period
AF = mybir.ActivationFunctionType
BF16 = mybir.dt.bfloat16
F32 = mybir.dt.float32
I32 = mybir.dt.int32

_CACHE = {}


def _fix_sync_waits(nc, max_waits=1):
    import bass_rust
    for fn in nc.m.functions:
        for bb in fn.blocks:
            out = []
            for inst in bb.instructions:
                si = inst.sync_info
                if si is not None and si.on_wait and len(si.on_wait) > max_waits:
                    waits = list(si.on_wait)
                    extra, keep = waits[:-max_waits], waits[-max_waits:]
                    for j in range(0, len(extra), max_waits):
                        nop = mybir.InstNoOp(name=f"{inst.name}_ws{j}", ins=[], outs=[])
                        nop.engine = inst.engine
                        nop.sync_info = bass_rust.SyncInfo(
                            on_wait=extra[j:j + max_waits], on_update=[])
                        out.append(nop)
                    inst.sync_info = bass_rust.SyncInfo(
                        on_wait=keep, on_update=list(si.on_update or []))
                out.append(inst)
            bb.instructions = out


def build(upto="full"):
    # upto: "ab" | "scan" | "enc" | "full" — truncation points for sim profiling
    nc = bass.Bass()
    dp = nc.declare_dram_parameter
    tok_in = dp("tok", [NGT, P], mybir.dt.uint16, isOutput=False)  # per-call
    emb_in = dp("emb", [V, E], BF16, isOutput=False)
    perm_in = dp("perm", [P, 2, P], BF16, isOutput=False)      # ident, rev16x8
    whh_in = dp("whh", [P, 2 * NG * NK, P], BF16, isOutput=False)
    wih_in = dp("wih", [P, 2 * NG * NE, P], BF16, isOutput=False)
    bias_in = dp("bias", [P, 2, NG], F32, isOutput=False)
    wenc_in = dp("wenc", [P, 2 * NK * NK, P], BF16, isOutput=False)
    benc_in = dp("benc", [P, NK], F32, isOutput=False)
    wout_in = dp("wout", [P, NK, K], BF16, isOutput=False)
    bout_in = dp("bout", [K, 1], F32, isOutput=False)
    pp_in = dp("pp", [K, K + 1], BF16, isOutput=False)
    a0_in = dp("a0", [K, CB], BF16, isOutput=False)
    # single packed output => one d2h sync per call (each fetched jax Array
    # pays the ~72ms tunnel round trip separately)
    res_out = dp("res", [1, CB * (T // ZR) + K * CB], F32, isOutput=True)

    xp_dram = [nc.dram_tensor(f"xp{d}", [NG, P, TB], BF16) for d in range(2)]

    with tile.TileContext(nc) as tc:
        with tc.tile_pool(name="persist", bufs=1) as pers:
            whh = pers.tile([P, 2 * NG * NK, P], BF16)
            nc.sync.dma_start(whh[:], whh_in[:])
            bias = pers.tile([P, 2, NG], F32)
            nc.sync.dma_start(bias[:], bias_in[:])
            hseq = [pers.tile([P, NK, TB], BF16, name=f"hseq{_d}") for _d in range(2)]

            # ---- phase A+B: gather + transpose + xp GEMMs ----
            with (
                tc.tile_pool(name="g1", bufs=1) as c1,
                tc.tile_pool(name="w1", bufs=3) as w1,
                tc.tile_pool(name="pt", bufs=2, space="PSUM") as pst,
                tc.tile_pool(name="p1", bufs=2, space="PSUM") as ps1,
            ):
                perm = c1.tile([P, 2, P], BF16)
                nc.sync.dma_start(perm[:], perm_in[:])
                tok16 = c1.tile([P, NGT], mybir.dt.uint16)
                nc.sync.dma_start(tok16[:], tok_in[:].rearrange("a p -> p a"))
                tok = c1.tile([P, NGT], I32)
                nc.vector.tensor_copy(tok[:], tok16[:])
                wih = c1.tile([P, 2 * NG * NE, P], BF16)
                nc.sync.dma_start(wih[:], wih_in[:])
                xT = [c1.tile([P, NE, TB], BF16, name=f"xT{_d}") for _d in range(2)]

                def xp_gemm(d, blk):
                    for m in range(NG):
                        ps = ps1.tile([P, 512], F32, tag="xps", name="ps")
                        for e in range(NE):
                            nc.tensor.matmul(
                                ps[:], lhsT=wih[:, (d * NG + m) * NE + e, :],
                                rhs=xT[d][:, e, blk * 512:(blk + 1) * 512],
                                start=(e == 0), stop=(e == NE - 1))
                        xo = w1.tile([P, 512], BF16, tag="xpo", name="xo")
                        nc.vector.tensor_scalar_add(
                            xo[:], ps[:], bias[:, d, m:m + 1])
                        nc.sync.dma_start(
                            xp_dram[d][m, :, blk * 512:(blk + 1) * 512], xo[:])

                # interleave the xp GEMMs with the gathers: each 4-group chunk
                # completes one 512-col block of xT_f (blk g//4) and one of
                # xT_r (blk 7-g//4), so its GEMMs can run while later groups
                # are still gathering
                for g in range(NGT):
                    emb_g = w1.tile([P, E], BF16, tag="g")
                    nc.gpsimd.indirect_dma_start(
                        out=emb_g[:], out_offset=None,
                        in_=emb_in[:],
                        in_offset=bass.IndirectOffsetOnAxis(
                            ap=tok[:, g:g + 1], axis=0),
                    )
                    for e in range(NE):
                        pt = pst.tile([P, P], BF16, tag="t")
                        nc.tensor.transpose(pt[:], emb_g[:, e * P:(e + 1) * P],
                                            perm[:, 0, :])
                        nc.vector.tensor_copy(xT[0][:, e, g * P:(g + 1) * P], pt[:])
                        pr = pst.tile([P, P], BF16, tag="r")
                        nc.tensor.transpose(pr[:], emb_g[:, e * P:(e + 1) * P],
                                            perm[:, 1, :])
                        nc.vector.tensor_copy(
                            xT[1][:, e, (NGT - 1 - g) * P:(NGT - g) * P], pr[:])
                    if g % 4 == 3:
                        xp_gemm(0, g // 4)
                        xp_gemm(1, NBLK - 1 - g // 4)

            # ---- phase C: interleaved fwd/bwd LSTM scans ----
            if upto == "ab":
                return _finish(nc)
            with (
                tc.tile_pool(name="c2", bufs=1) as c2,
                tc.tile_pool(name="p2", bufs=2, space="PSUM") as ps2,
            ):
                xr = [c2.tile([P, 2, NG, GRP, BS], BF16, name=f"xr{_d}") for _d in range(2)]
                h0 = c2.tile([P, NK, BS], BF16)
                nc.any.memset(h0[:], 0.0)
                c_t = [c2.tile([P, NK * BS], F32, name=f"c_t{_d}") for _d in range(2)]
                gs = [c2.tile([P, NG * BS], F32, name=f"gs{_d}") for _d in range(2)]
                sio = [c2.tile([P, 3 * NK * BS], F32, name=f"sio{_d}") for _d in range(2)]
                tg = [c2.tile([P, NK * BS], F32, name=f"tg{_d}") for _d in range(2)]
                m1 = [c2.tile([P, NK * BS], F32, name=f"m1_{_d}") for _d in range(2)]
                m2 = [c2.tile([P, NK * BS], F32, name=f"m2_{_d}") for _d in range(2)]
                tcc = [c2.tile([P, NK * BS], F32, name=f"tcc{_d}") for _d in range(2)]
                for d in range(2):
                    nc.any.memset(c_t[d][:], 0.0)

                def prefetch(d, g):
                    if g >= NGRP:
                        return
                    for m in range(NG):
                        nc.sync.dma_start(
                            xr[d][:, g % 2, m, :, :].rearrange("p t b -> p (t b)"),
                            xp_dram[d][m, :, g * GRP * BS:(g + 1) * GRP * BS])

                for d in range(2):
                    prefetch(d, 0)
                    prefetch(d, 1)

                def seq_col(d, i):
                    # column range in hseq[d] written at scan step i
                    at = i if d == 0 else T - 1 - i
                    return slice(at * BS, (at + 1) * BS)

                for i in range(T):
                    g, tm = divmod(i, GRP)
                    gp = []
                    for d in range(2):
                        hin = (h0[:] if i == 0
                               else hseq[d][:, :, seq_col(d, i - 1)])
                        gpd = ps2.tile([P, NG * BS], F32, tag=f"g{d}")
                        for m in range(NG):
                            for k in range(NK):
                                nc.tensor.matmul(
                                    gpd[:, m * BS:(m + 1) * BS],
                                    lhsT=whh[:, (d * NG + m) * NK + k, :],
                                    rhs=hin[:, k, :],
                                    start=(k == 0), stop=(k == NK - 1))
                        gp.append(gpd)
                    for d in range(2):
                        nc.vector.tensor_tensor(
                            gs[d][:].rearrange("p (m b) -> p m b", b=BS),
                            gp[d][:].rearrange("p (m b) -> p m b", b=BS),
                            xr[d][:, g % 2, :, tm, :], mybir.AluOpType.add)
                        nc.scalar.activation(sio[d][:], gs[d][:, 0:3 * NK * BS],
                                             AF.Sigmoid)
                        nc.scalar.activation(tg[d][:], gs[d][:, 3 * NK * BS:],
                                             AF.Tanh)
                        nc.vector.tensor_mul(m1[d][:], sio[d][:, 0:NK * BS], tg[d][:])
                        nc.vector.tensor_mul(m2[d][:], sio[d][:, NK * BS:2 * NK * BS],
                                             c_t[d][:])
                        nc.vector.tensor_add(c_t[d][:], m1[d][:], m2[d][:])
                        nc.scalar.activation(tcc[d][:], c_t[d][:], AF.Tanh)
                        nc.vector.tensor_mul(
                            hseq[d][:, :, seq_col(d, i)],
                            sio[d][:, 2 * NK * BS:3 * NK * BS].rearrange(
                                "p (a b) -> p a b", b=BS),
                            tcc[d][:].rearrange("p (a b) -> p a b", b=BS))
                    if tm == GRP - 1:
                        for d in range(2):
                            prefetch(d, g + 2)

            # ---- phase D: encoder + emissions;  phase E: CRF scan ----
            if upto == "scan":
                return _finish(nc)
            with tc.tile_pool(name="c3", bufs=1) as c3:
                ps3_cm = tc.tile_pool(name="p3", bufs=2, space="PSUM")
                ps3 = ps3_cm.__enter__()
                wenc = c3.tile([P, 2 * NK * NK, P], BF16)
                nc.sync.dma_start(wenc[:], wenc_in[:])
                benc = c3.tile([P, NK], F32)
                nc.sync.dma_start(benc[:], benc_in[:])
                wout = c3.tile([P, NK, K], BF16)
                nc.sync.dma_start(wout[:], wout_in[:])
                bout = c3.tile([K, 1], F32)
                nc.sync.dma_start(bout[:], bout_in[:])
                states = c3.tile([P, NK, TB], BF16)

                for blk in range(NBLK):
                    sl = slice(blk * 512, (blk + 1) * 512)
                    for m in range(NK):
                        ps = ps3.tile([P, 512], F32, tag="enc")
                        for k in range(NK):
                            nc.tensor.matmul(ps[:], lhsT=wenc[:, m * NK + k, :],
                                             rhs=hseq[0][:, k, sl],
                                             start=(k == 0), stop=False)
                        for k in range(NK):
                            nc.tensor.matmul(ps[:], lhsT=wenc[:, 16 + m * NK + k, :],
                                             rhs=hseq[1][:, k, sl], start=False,
                                             stop=(k == NK - 1))
                        nc.scalar.activation(states[:, m, sl], ps[:], AF.Tanh,
                                             bias=benc[:, m:m + 1])

                expE = c3.tile([K, TB], F32)
                for blk in range(NBLK):
                    sl = slice(blk * 512, (blk + 1) * 512)
                    ps = ps3.tile([K, 512], F32, tag="emit")
                    for k in range(NK):
                        nc.tensor.matmul(ps[:], lhsT=wout[:, k, :],
                                         rhs=states[:, k, sl],
                                         start=(k == 0), stop=(k == NK - 1))
                    nc.scalar.activation(expE[:, sl], ps[:], AF.Exp,
                                         bias=bout[:, 0:1])

                ps3_cm.__exit__(None, None, None)
                if upto == "enc":
                    return _finish(nc)
                psE_cm = tc.tile_pool(name="pE", bufs=1, space="PSUM")
                psE = psE_cm.__enter__()
                pp = c3.tile([K, K + 1], BF16)
                nc.sync.dma_start(pp[:], pp_in[:])
                ones_r = c3.tile([1, K], BF16)
                nc.any.memset(ones_r[:], 1.0)
                A = c3.tile([K, CB], BF16)
                nc.sync.dma_start(A[:], a0_in[:])
                zbuf = c3.tile([1, CB, T // ZR], F32)
                izb = c3.tile([1, CB], F32)
                izb_bf = c3.tile([1, CB], BF16)

                # two independent batch halves: half 1's matmul runs on PE
                # while half 0's emission multiply runs on DVE, hiding the
                # PE<->DVE semaphore latency of the sequential chain
                HB = CB // 2
                for t in range(T):
                    pss = []
                    for h in range(2):
                        hs = slice(h * HB, (h + 1) * HB)
                        ps = psE.tile([K, HB], F32, tag=f"crf{h}", bufs=2,
                                      name="ps")
                        nc.tensor.matmul(ps[:], lhsT=pp[:, 0:K], rhs=A[:, hs],
                                         start=True, stop=True)
                        pss.append(ps)
                    if t % ZR == ZR - 1:
                        r = t // ZR
                        for h in range(2):
                            hs = slice(h * HB, (h + 1) * HB)
                            zps = psE.tile([1, HB], F32, tag="zps", bufs=1,
                                           name="zps")
                            nc.tensor.matmul(zps[:], lhsT=pp[:, K:K + 1],
                                             rhs=A[:, hs], start=True, stop=True)
                            nc.vector.tensor_copy(zbuf[:, hs, r], zps[:])
                            nc.vector.reciprocal(izb[:, hs], zps[:])
                            nc.vector.tensor_copy(izb_bf[:, hs], izb[:, hs])
                            zb = psE.tile([K, HB], F32, tag="zbc", bufs=1,
                                          name="zb")
                            nc.tensor.matmul(zb[:], lhsT=ones_r[:],
                                             rhs=izb_bf[:, hs],
                                             start=True, stop=True)
                            nc.vector.tensor_mul(A[:, hs], pss[h][:],
                                                 expE[:, t * CB + h * HB:
                                                      t * CB + (h + 1) * HB])
                            nc.vector.tensor_mul(A[:, hs], A[:, hs], zb[:])
                    else:
                        for h in range(2):
                            hs = slice(h * HB, (h + 1) * HB)
                            nc.vector.tensor_mul(A[:, hs], pss[h][:],
                                                 expE[:, t * CB + h * HB:
                                                      t * CB + (h + 1) * HB])

                psE_cm.__exit__(None, None, None)
                nzb = CB * (T // ZR)
                nc.sync.dma_start(
                    res_out[:, 0:nzb],
                    zbuf[:].rearrange("p a b -> p (a b)"))
                af = c3.tile([K, CB], F32)
                nc.vector.tensor_copy(af[:], A[:])
                nc.sync.dma_start(
                    res_out[0, nzb:].rearrange("(k c) -> k c", c=CB), af[:])

    return _finish(nc)


def _finish(nc):
    _fix_sync_waits(nc)
    return nc


# ---------------- cached PJRT runner ----------------

class _Runner:
    """jit-compile the SPMD callable once; keep weights device-resident.
    (The stock run_bass_kernel_spmd builds a fresh closure per call => full
    retrace + XLA recompile, ~9s/call on this runtime.)"""

    def __init__(self, nc, n_cores):
        import jax
        from jax.sharding import Mesh, PartitionSpec, NamedSharding
        from jax.experimental.shard_map import shard_map
        from concourse.bass2jax import (_bass_exec_p, partition_id_tensor,
                                        install_neuronx_cc_hook)
        install_neuronx_cc_hook()
        self.jax = jax
        self.n_cores = n_cores
        partition_name = (nc.partition_id_tensor.name
                          if nc.partition_id_tensor else None)
        in_names, out_names, out_avals, zero_outs = [], [], [], []
        for alloc in nc.m.functions[0].allocations:
            if not isinstance(alloc, mybir.MemoryLocationSet):
                continue
            name = alloc.memorylocations[0].name
            if alloc.kind == "ExternalInput":
                if name != partition_name:
                    in_names.append(name)
            elif alloc.kind == "ExternalOutput":
                out_names.append(name)
                shape = tuple(alloc.tensor_shape)
                dtype = mybir.dt.np(alloc.dtype)
                out_avals.append(jax.core.ShapedArray(shape, dtype))
                zero_outs.append(np.zeros(shape, dtype))
        self.in_names = in_names
        self.out_names = out_names
        self.out_avals = out_avals

        def _body(*args):
            operands = list(args)
            if partition_name is not None:
                operands.append(partition_id_tensor())
            outs = _bass_exec_p.bind(
                *operands,
                out_avals=tuple(out_avals),
                in_names=tuple(in_names + out_names
                               + ([partition_name] if partition_name else [])),
                out_names=tuple(out_names),
                lowering_input_output_aliases=(),
                sim_require_finite=True,
                sim_require_nnan=True,
                nc=nc,
            )
            return tuple(outs)

        devices = jax.devices()[:n_cores]
        mesh = Mesh(np.asarray(devices), ("core",))
        self.sharding = NamedSharding(mesh, PartitionSpec("core"))
        n_io = len(in_names) + len(out_names)
        self.fn = jax.jit(
            shard_map(_body, mesh=mesh,
                      in_specs=(PartitionSpec("core"),) * n_io,
                      out_specs=(PartitionSpec("core"),) * len(out_names),
                      check_rep=False),
            keep_unused=True,
        )
        self.dev_zeros = [
            jax.device_put(np.zeros((n_cores * z.shape[0], *z.shape[1:]),
                                    z.dtype), self.sharding)
            for z in zero_outs
        ]
        self.resident = {}

    def put_resident(self, name, per_core_arrays):
        cat = np.concatenate([np.asarray(a) for a in per_core_arrays], axis=0)
        self.resident[name] = self.jax.device_put(cat, self.sharding)

    def dispatch(self, **per_call):
        """Async: returns un-fetched global output arrays."""
        args = []
        for name in self.in_names:
            if name in per_call:
                v = per_call[name]
                if isinstance(v, (list, tuple)):
                    v = np.concatenate([np.asarray(a) for a in v], axis=0)
                args.append(self.jax.device_put(v, self.sharding))
            else:
                args.append(self.resident[name])
        return self.fn(*args, *self.dev_zeros)

    def fetch(self, outs):
        host = [np.asarray(o) for o in outs]
        return [
            {name: host[i].reshape(self.n_cores, *self.out_avals[i].shape)[c]
             for i, name in enumerate(self.out_names)}
            for c in range(self.n_cores)
        ]

    def __call__(self, **per_call):
        return self.fetch(self.dispatch(**per_call))


# ---------------- host-side prep ----------------

GPERM = np.concatenate([
    np.arange(0, 512), np.arange(512, 1024),
    np.arange(1536, 2048), np.arange(1024, 1536)])  # i,f,o,g tile order


def _prep_weights(w_ih_f, w_hh_f, b_f, w_ih_b, w_hh_b, b_b,
                  w_enc, b_enc, w_out, b_out, trans, embed):
    bf = ml_dtypes.bfloat16
    whh_t = np.empty((P, 2 * NG * NK, P), np.float32)
    wih_t = np.empty((P, 2 * NG * NE, P), np.float32)
    bias = np.empty((P, 2, NG), np.float32)
    for d, (wih_d, whh_d, b_d) in enumerate(
            [(w_ih_f, w_hh_f, b_f), (w_ih_b, w_hh_b, b_b)]):
        Whh = whh_d[GPERM]
        Wih = wih_d[GPERM]
        for m in range(NG):
            for k in range(NK):
                whh_t[:, (d * NG + m) * NK + k, :] = \
                    Whh[m * P:(m + 1) * P, k * P:(k + 1) * P].T
            for e in range(NE):
                wih_t[:, (d * NG + m) * NE + e, :] = \
                    Wih[m * P:(m + 1) * P, e * P:(e + 1) * P].T
        bias[:, d, :] = b_d[GPERM].reshape(NG, P).T

    wenc_t = np.empty((P, 2 * NK * NK, P), np.float32)
    for dd in range(2):
        Wd = w_enc[:, dd * H:(dd + 1) * H]
        for m in range(NK):
            for k in range(NK):
                wenc_t[:, dd * 16 + m * NK + k, :] = \
                    Wd[m * P:(m + 1) * P, k * P:(k + 1) * P].T
    benc_t = b_enc.reshape(NK, P).T.copy()
    wout_t = np.empty((P, NK, K), np.float32)
    for k in range(NK):
        wout_t[:, k, :] = w_out[:, k * P:(k + 1) * P].T

    pp = np.zeros((K, K + 1), np.float32)
    pp[:, :K] = np.exp(trans.astype(np.float64)).T.astype(np.float32)
    pp[:, K] = 1.0
    a0 = np.zeros((K, CB), np.float32)
    a0[0, :] = 1.0

    ident = np.eye(P, dtype=np.float32)
    rev = np.zeros((P, P), np.float32)
    tt, bb_ = np.meshgrid(np.arange(P // BS), np.arange(BS), indexing='ij')
    rev[(P // BS - 1 - tt) * BS + bb_, tt * BS + bb_] = 1.0
    perm = np.stack([ident, rev], axis=1)

    return {
        "emb": embed.astype(bf),
        "perm": perm.astype(bf),
        "whh": whh_t.astype(bf), "wih": wih_t.astype(bf),
        "bias": bias,
        "wenc": wenc_t.astype(bf), "benc": benc_t,
        "wout": wout_t.astype(bf), "bout": b_out.reshape(K, 1).astype(np.float32),
        "pp": pp.astype(bf), "a0": a0.astype(bf),
    }


def _weights_sig(arrs):
    h = hashlib.blake2b(digest_size=16)
    for a in arrs:
        a = np.asarray(a)
        h.update(str(a.shape).encode())
        if a.nbytes <= 1 << 20:
            h.update(np.ascontiguousarray(a).tobytes())
        else:
            h.update(np.ascontiguousarray(a.reshape(-1)[::17]).tobytes())
    return h.digest()


def _put_weights(runner, wlist, sig):
    prepped = _prep_weights(*[np.asarray(w, dtype=np.float32)
                              for w in wlist[1:]],
                            np.asarray(wlist[0], dtype=np.float32))
    for name, arr in prepped.items():
        runner.put_resident(name, [arr] * 8)
    _CACHE["sig"] = sig
    _CACHE["etstop"] = np.exp(np.asarray(wlist[-1])[K - 1].astype(np.float64))


def kernel(tokens, embed, w_ih_f, w_hh_f, b_f, w_ih_b, w_hh_b, b_b,
           w_enc, b_enc, w_out, b_out, trans):
    tokens = np.asarray(tokens)
    if tokens.dtype != np.int32:
        tokens = tokens.astype(np.int32)
    wlist = [embed, w_ih_f, w_hh_f, b_f, w_ih_b, w_hh_b, b_b,
             w_enc, b_enc, w_out, b_out, trans]

    if "runner" not in _CACHE:
        _CACHE["runner"] = _Runner(build(), 8)
        _put_weights(_CACHE["runner"], wlist, _weights_sig(wlist))
    runner = _CACHE["runner"]

    toks = [np.ascontiguousarray(tokens[:, c * BS:(c + 1) * BS])
            .reshape(NGT, P).astype(np.uint16) for c in range(8)]
    # dispatch optimistically with the resident weights, then verify the
    # weight signature while the device runs; redo on mismatch (rare).
    # Retry on transient proxy/transport errors.
    last_err = None
    for attempt in range(3):
        try:
            outs = runner.dispatch(tok=toks)
            sig = _weights_sig(wlist)
            if _CACHE["sig"] != sig:
                _put_weights(runner, wlist, sig)
                outs = runner.dispatch(tok=toks)
            res = runner.fetch(outs)
            break
        except Exception as e:  # noqa: BLE001 - re-raised after retries
            last_err = e
            import time as _time
            _time.sleep(0.5 * (attempt + 1))
    else:
        raise last_err

    etstop = _CACHE["etstop"]
    nzb = CB * (T // ZR)
    out = np.empty((B,), np.float32)
    for c in range(8):
        flat = np.asarray(res[c]["res"], dtype=np.float64).reshape(-1)
        zb = flat[:nzb].reshape(CB, T // ZR)
        af = flat[nzb:].reshape(K, CB)
        lz = np.log(zb).sum(axis=1) + np.log(etstop @ af)
        out[c * BS:(c + 1) * BS] = lz.astype(np.float32)
    return out


# revision 20
# speedup vs baseline: 2.2180x; 1.1212x over previous
"""BiLSTM-CRF forward (log partition) on 8 trn2 NeuronCores — single fused
SPMD kernel, one launch per call.

Sharding: data-parallel over batch. Each core owns 8 of the 64 batch columns
and runs the whole pipeline for them: embedding gather (indirect DMA from a
device-resident table), input-projection GEMMs, forward+backward LSTM scans
(interleaved per step for engine overlap), encoder GEMM, emissions, and the
CRF forward scan in the exp domain with periodic rescaling. Outputs per core
are tiny (the 32 rescale factors and the final alpha vector); the host sums
logs and applies the STOP transition.

Per-call host<->device traffic is ~150KB (the axon tunnel runs ~35-60MB/s, so
the baseline's 125MB/call dominated). All weights and the embedding table are
transferred once and kept device-resident; the jitted PJRT callable is built
once and cached (a fresh closure per call costs ~9s in retrace/recompile).
Per-call wall time is bound by the tunnel's per-sync round trip (~52-125ms
depending on network phase); device execution (~1.5ms sim: scan 1.05ms,
A+B 0.2ms, CRF 0.17ms, encoder 0.07ms) pipelines entirely inside that sync
(slope-test: 9 pipelined executes cost no more wall than 1). The xp GEMMs are
interleaved into the gather loop and the CRF runs as two independent batch
halves so PE/DVE dependency chains overlap.
"""
import hashlib
import numpy as np
import ml_dtypes

import concourse.bass as bass
import concourse.mybir as mybir
import concourse.tile as tile

T, B, E, H, V, K = 512, 64, 256, 512, 50000, 50
BS = 8             # batch per core
P = 128
NG = 16            # gate tiles (4H/128)
NK = 4             # h chunks (H/128)
NE = 2             # e chunks (E/128)
GRP = 32           # scan steps per xp prefetch group
NGRP = T // GRP    # 16
TB = T * BS        # 4096
NBLK = TB // 512   # 8
NGT = TB // P      # 32 token groups of 128
CB = BS            # CRF batch per core
ZR = 16            # CRF rescale period
AF = mybir.ActivationFunctionType
BF16 = mybir.dt.bfloat16
F32 = mybir.dt.float32
I32 = mybir.dt.int32

_CACHE = {}


def _fix_sync_waits(nc, max_waits=1):
    import bass_rust
    for fn in nc.m.functions:
        for bb in fn.blocks:
            out = []
            for inst in bb.instructions:
                si = inst.sync_info
                if si is not None and si.on_wait and len(si.on_wait) > max_waits:
                    waits = list(si.on_wait)
                    extra, keep = waits[:-max_waits], waits[-max_waits:]
                    for j in range(0, len(extra), max_waits):
                        nop = mybir.InstNoOp(name=f"{inst.name}_ws{j}", ins=[], outs=[])
                        nop.engine = inst.engine
                        nop.sync_info = bass_rust.SyncInfo(
                            on_wait=extra[j:j + max_waits], on_update=[])
                        out.append(nop)
                    inst.sync_info = bass_rust.SyncInfo(
                        on_wait=keep, on_update=list(si.on_update or []))
                out.append(inst)
            bb.instructions = out


def build(upto="full"):
    # upto: "ab" | "scan" | "enc" | "full" — truncation points for sim profiling
    nc = bass.Bass()
    dp = nc.declare_dram_parameter
    tok_in = dp("tok", [NGT, P], mybir.dt.uint16, isOutput=False)  # per-call
    emb_in = dp("emb", [V, E], BF16, isOutput=False)
    perm_in = dp("perm", [P, 2, P], BF16, isOutput=False)      # ident, rev16x8
    whh_in = dp("whh", [P, 2 * NG * NK, P], BF16, isOutput=False)
    wih_in = dp("wih", [P, 2 * NG * NE, P], BF16, isOutput=False)
    bias_in = dp("bias", [P, 2, NG], F32, isOutput=False)
    wenc_in = dp("wenc", [P, 2 * NK * NK, P], BF16, isOutput=False)
    benc_in = dp("benc", [P, NK], F32, isOutput=False)
    wout_in = dp("wout", [P, NK, K], BF16, isOutput=False)
    bout_in = dp("bout", [K, 1], F32, isOutput=False)
    pp_in = dp("pp", [K, K + 1], BF16, isOutput=False)
    a0_in = dp("a0", [K, CB], BF16, isOutput=False)
    # single packed output => one d2h sync per call (each fetched jax Array
    # pays the ~72ms tunnel round trip separately)
    res_out = dp("res", [1, CB * (T // ZR) + K * CB], F32, isOutput=True)

    xp_dram = [nc.dram_tensor(f"xp{d}", [NG, P, TB], BF16) for d in range(2)]

    with tile.TileContext(nc) as tc:
        with tc.tile_pool(name="persist", bufs=1) as pers:
            whh = pers.tile([P, 2 * NG * NK, P], BF16)
            nc.sync.dma_start(whh[:], whh_in[:])
            bias = pers.tile([P, 2, NG], F32)
            nc.sync.dma_start(bias[:], bias_in[:])
            hseq = [pers.tile([P, NK, TB], BF16, name=f"hseq{_d}") for _d in range(2)]

            # ---- phase A+B: gather + transpose + xp GEMMs ----
            with (
                tc.tile_pool(name="g1", bufs=1) as c1,
                tc.tile_pool(name="w1", bufs=3) as w1,
                tc.tile_pool(name="pt", bufs=2, space="PSUM") as pst,
                tc.tile_pool(name="p1", bufs=2, space="PSUM") as ps1,
            ):
                perm = c1.tile([P, 2, P], BF16)
                nc.sync.dma_start(perm[:], perm_in[:])
                tok16 = c1.tile([P, NGT], mybir.dt.uint16)
                nc.sync.dma_start(tok16[:], tok_in[:].rearrange("a p -> p a"))
                tok = c1.tile([P, NGT], I32)
                nc.vector.tensor_copy(tok[:], tok16[:])
                wih = c1.tile([P, 2 * NG * NE, P], BF16)
                nc.sync.dma_start(wih[:], wih_in[:])
                xT = [c1.tile([P, NE, TB], BF16, name=f"xT{_d}") for _d in range(2)]

                def xp_gemm(d, blk):
                    for m in range(NG):
                        ps = ps1.tile([P, 512], F32, tag="xps", name="ps")
                        for e in range(NE):
                            nc.tensor.matmul(
                                ps[:], lhsT=wih[:, (d * NG + m) * NE + e, :],
                                rhs=xT[d][:, e, blk * 512:(blk + 1) * 512],
                                start=(e == 0), stop=(e == NE - 1))
                        xo = w1.tile([P, 512], BF16, tag="xpo", name="xo")
                        nc.vector.tensor_scalar_add(
                            xo[:], ps[:], bias[:, d, m:m + 1])
                        nc.sync.dma_start(
                            xp_dram[d][m, :, blk * 512:(blk + 1) * 512], xo[:])

                # interleave the xp GEMMs with the gathers: each 4-group chunk
                # completes one 512-col block of xT_f (blk g//4) and one of
                # xT_r (blk 7-g//4), so its GEMMs can run while later groups
                # are still gathering
                for g in range(NGT):
                    emb_g = w1.tile([P, E], BF16, tag="g")
                    nc.gpsimd.indirect_dma_start(
                        out=emb_g[:], out_offset=None,
                        in_=emb_in[:],
                        in_offset=bass.IndirectOffsetOnAxis(
                            ap=tok[:, g:g + 1], axis=0),
                    )
                    for e in range(NE):
                        pt = pst.tile([P, P], BF16, tag="t")
                        nc.tensor.transpose(pt[:], emb_g[:, e * P:(e + 1) * P],
                                            perm[:, 0, :])
                        nc.vector.tensor_copy(xT[0][:, e, g * P:(g + 1) * P], pt[:])
                        pr = pst.tile([P, P], BF16, tag="r")
                        nc.tensor.transpose(pr[:], emb_g[:, e * P:(e + 1) * P],
                                            perm[:, 1, :])
                        nc.vector.tensor_copy(
                            xT[1][:, e, (NGT - 1 - g) * P:(NGT - g) * P], pr[:])
                    if g % 4 == 3:
                        xp_gemm(0, g // 4)
                        xp_gemm(1, NBLK - 1 - g // 4)

            # ---- phase C: interleaved fwd/bwd LSTM scans ----
            if upto == "ab":
                return _finish(nc)
            with (
                tc.tile_pool(name="c2", bufs=1) as c2,
                tc.tile_pool(name="p2", bufs=2, space="PSUM") as ps2,
            ):
                xr = [c2.tile([P, 2, NG, GRP, BS], BF16, name=f"xr{_d}") for _d in range(2)]
                h0 = c2.tile([P, NK, BS], BF16)
                nc.any.memset(h0[:], 0.0)
                c_t = [c2.tile([P, NK * BS], F32, name=f"c_t{_d}") for _d in range(2)]
                gs = [c2.tile([P, NG * BS], F32, name=f"gs{_d}") for _d in range(2)]
                sio = [c2.tile([P, 3 * NK * BS], F32, name=f"sio{_d}") for _d in range(2)]
                tg = [c2.tile([P, NK * BS], F32, name=f"tg{_d}") for _d in range(2)]
                m1 = [c2.tile([P, NK * BS], F32, name=f"m1_{_d}") for _d in range(2)]
                m2 = [c2.tile([P, NK * BS], F32, name=f"m2_{_d}") for _d in range(2)]
                tcc = [c2.tile([P, NK * BS], F32, name=f"tcc{_d}") for _d in range(2)]
                for d in range(2):
                    nc.any.memset(c_t[d][:], 0.0)

                def prefetch(d, g):
                    if g >= NGRP:
                        return
                    for m in range(NG):
                        nc.sync.dma_start(
                            xr[d][:, g % 2, m, :, :].rearrange("p t b -> p (t b)"),
                            xp_dram[d][m, :, g * GRP * BS:(g + 1) * GRP * BS])

                for d in range(2):
                    prefetch(d, 0)
                    prefetch(d, 1)

                def seq_col(d, i):
                    # column range in hseq[d] written at scan step i
                    at = i if d == 0 else T - 1 - i
                    return slice(at * BS, (at + 1) * BS)

                for i in range(T):
                    g, tm = divmod(i, GRP)
                    gp = []
                    for d in range(2):
                        hin = (h0[:] if i == 0
                               else hseq[d][:, :, seq_col(d, i - 1)])
                        gpd = ps2.tile([P, NG * BS], F32, tag=f"g{d}")
                        for m in range(NG):
                            for k in range(NK):
                                nc.tensor.matmul(
                                    gpd[:, m * BS:(m + 1) * BS],
                                    lhsT=whh[:, (d * NG + m) * NK + k, :],
                                    rhs=hin[:, k, :],
                                    start=(k == 0), stop=(k == NK - 1))
                        gp.append(gpd)
                    for d in range(2):
                        nc.vector.tensor_tensor(
                            gs[d][:].rearrange("p (m b) -> p m b", b=BS),
                            gp[d][:].rearrange("p (m b) -> p m b", b=BS),
                            xr[d][:, g % 2, :, tm, :], mybir.AluOpType.add)
                        nc.scalar.activation(sio[d][:], gs[d][:, 0:3 * NK * BS],
                                             AF.Sigmoid)
                        nc.scalar.activation(tg[d][:], gs[d][:, 3 * NK * BS:],
                                             AF.Tanh)
                        nc.vector.tensor_mul(m1[d][:], sio[d][:, 0:NK * BS], tg[d][:])
                        nc.vector.tensor_mul(m2[d][:], sio[d][:, NK * BS:2 * NK * BS],
                                             c_t[d][:])
                        nc.vector.tensor_add(c_t[d][:], m1[d][:], m2[d][:])
                        nc.scalar.activation(tcc[d][:], c_t[d][:], AF.Tanh)
                        nc.vector.tensor_mul(
                            hseq[d][:, :, seq_col(d, i)],
                            sio[d][:, 2 * NK * BS:3 * NK * BS].rearrange(
                                "p (a b) -> p a b", b=BS),
                            tcc[d][:].rearrange("p (a b) -> p a b", b=BS))
                    if tm == GRP - 1:
                        for d in range(2):
                            prefetch(d, g + 2)

            # ---- phase D: encoder + emissions;  phase E: CRF scan ----
            if upto == "scan":
                return _finish(nc)
            with tc.tile_pool(name="c3", bufs=1) as c3:
                ps3_cm = tc.tile_pool(name="p3", bufs=2, space="PSUM")
                ps3 = ps3_cm.__enter__()
                wenc = c3.tile([P, 2 * NK * NK, P], BF16)
                nc.sync.dma_start(wenc[:], wenc_in[:])
                benc = c3.tile([P, NK], F32)
                nc.sync.dma_start(benc[:], benc_in[:])
                wout = c3.tile([P, NK, K], BF16)
                nc.sync.dma_start(wout[:], wout_in[:])
                bout = c3.tile([K, 1], F32)
                nc.sync.dma_start(bout[:], bout_in[:])
                states = c3.tile([P, NK, TB], BF16)

                for blk in range(NBLK):
                    sl = slice(blk * 512, (blk + 1) * 512)
                    for m in range(NK):
                        ps = ps3.tile([P, 512], F32, tag="enc")
                        for k in range(NK):
                            nc.tensor.matmul(ps[:], lhsT=wenc[:, m * NK + k, :],
                                             rhs=hseq[0][:, k, sl],
                                             start=(k == 0), stop=False)
                        for k in range(NK):
                            nc.tensor.matmul(ps[:], lhsT=wenc[:, 16 + m * NK + k, :],
                                             rhs=hseq[1][:, k, sl], start=False,
                                             stop=(k == NK - 1))
                        nc.scalar.activation(states[:, m, sl], ps[:], AF.Tanh,
                                             bias=benc[:, m:m + 1])

                expE = c3.tile([K, TB], F32)
                for blk in range(NBLK):
                    sl = slice(blk * 512, (blk + 1) * 512)
                    ps = ps3.tile([K, 512], F32, tag="emit")
                    for k in range(NK):
                        nc.tensor.matmul(ps[:], lhsT=wout[:, k, :],
                                         rhs=states[:, k, sl],
                                         start=(k == 0), stop=(k == NK - 1))
                    nc.scalar.activation(expE[:, sl], ps[:], AF.Exp,
                                         bias=bout[:, 0:1])

                ps3_cm.__exit__(None, None, None)
                if upto == "enc":
                    return _finish(nc)
                psE_cm = tc.tile_pool(name="pE", bufs=1, space="PSUM")
                psE = psE_cm.__enter__()
                pp = c3.tile([K, K + 1], BF16)
                nc.sync.dma_start(pp[:], pp_in[:])
                ones_r = c3.tile([1, K], BF16)
                nc.any.memset(ones_r[:], 1.0)
                A = c3.tile([K, CB], BF16)
                nc.sync.dma_start(A[:], a0_in[:])
                zbuf = c3.tile([1, CB, T // ZR], F32)
                izb = c3.tile([1, CB], F32)
                izb_bf = c3.tile([1, CB], BF16)

                # two independent batch halves: half 1's matmul runs on PE
                # while half 0's emission multiply runs on DVE, hiding the
                # PE<->DVE semaphore latency of the sequential chain
                HB = CB // 2
                for t in range(T):
                    pss = []
                    for h in range(2):
                        hs = slice(h * HB, (h + 1) * HB)
                        ps = psE.tile([K, HB], F32, tag=f"crf{h}", bufs=2,
                                      name="ps")
                        nc.tensor.matmul(ps[:], lhsT=pp[:, 0:K], rhs=A[:, hs],
                                         start=True, stop=True)
                        pss.append(ps)
                    if t % ZR == ZR - 1:
                        r = t // ZR
                        for h in range(2):
                            hs = slice(h * HB, (h + 1) * HB)
                            zps = psE.tile([1, HB], F32, tag="zps", bufs=1,
                                           name="zps")
                            nc.tensor.matmul(zps[:], lhsT=pp[:, K:K + 1],
                                             rhs=A[:, hs], start=True, stop=True)
                            nc.vector.tensor_copy(zbuf[:, hs, r], zps[:])
                            nc.vector.reciprocal(izb[:, hs], zps[:])
                            nc.vector.tensor_copy(izb_bf[:, hs], izb[:, hs])
                            zb = psE.tile([K, HB], F32, tag="zbc", bufs=1,
                                          name="zb")
                            nc.tensor.matmul(zb[:], lhsT=ones_r[:],
                                             rhs=izb_bf[:, hs],
                                             start=True, stop=True)
                            nc.vector.tensor_mul(A[:, hs], pss[h][:],
                                                 expE[:, t * CB + h * HB:
                                                      t * CB + (h + 1) * HB])
                            nc.vector.tensor_mul(A[:, hs], A[:, hs], zb[:])
                    else:
                        for h in range(2):
                            hs = slice(h * HB, (h + 1) * HB)
                            nc.vector.tensor_mul(A[:, hs], pss[h][:],
                                                 expE[:, t * CB + h * HB:
                                                      t * CB + (h + 1) * HB])

                psE_cm.__exit__(None, None, None)
                nzb = CB * (T // ZR)
                nc.sync.dma_start(
                    res_out[:, 0:nzb],
                    zbuf[:].rearrange("p a b -> p (a b)"))
                af = c3.tile([K, CB], F32)
                nc.vector.tensor_copy(af[:], A[:])
                nc.sync.dma_start(
                    res_out[0, nzb:].rearrange("(k c) -> k c", c=CB), af[:])

    return _finish(nc)


def _finish(nc):
    _fix_sync_waits(nc)
    return nc


# ---------------- cached PJRT runner ----------------

class _Runner:
    """jit-compile the SPMD callable once; keep weights device-resident.
    (The stock run_bass_kernel_spmd builds a fresh closure per call => full
    retrace + XLA recompile, ~9s/call on this runtime.)"""

    def __init__(self, nc, n_cores):
        import jax
        from jax.sharding import Mesh, PartitionSpec, NamedSharding
        from jax.experimental.shard_map import shard_map
        from concourse.bass2jax import (_bass_exec_p, partition_id_tensor,
                                        install_neuronx_cc_hook)
        install_neuronx_cc_hook()
        self.jax = jax
        self.n_cores = n_cores
        partition_name = (nc.partition_id_tensor.name
                          if nc.partition_id_tensor else None)
        in_names, out_names, out_avals, zero_outs = [], [], [], []
        for alloc in nc.m.functions[0].allocations:
            if not isinstance(alloc, mybir.MemoryLocationSet):
                continue
            name = alloc.memorylocations[0].name
            if alloc.kind == "ExternalInput":
                if name != partition_name:
                    in_names.append(name)
            elif alloc.kind == "ExternalOutput":
                out_names.append(name)
                shape = tuple(alloc.tensor_shape)
                dtype = mybir.dt.np(alloc.dtype)
                out_avals.append(jax.core.ShapedArray(shape, dtype))
                zero_outs.append(np.zeros(shape, dtype))
        self.in_names = in_names
        self.out_names = out_names
        self.out_avals = out_avals

        def _body(*args):
            operands = list(args)
            if partition_name is not None:
                operands.append(partition_id_tensor())
            outs = _bass_exec_p.bind(
                *operands,
                out_avals=tuple(out_avals),
                in_names=tuple(in_names + out_names
                               + ([partition_name] if partition_name else [])),
                out_names=tuple(out_names),
                lowering_input_output_aliases=(),
                sim_require_finite=True,
                sim_require_nnan=True,
                nc=nc,
            )
            return tuple(outs)

        devices = jax.devices()[:n_cores]
        mesh = Mesh(np.asarray(devices), ("core",))
        self.sharding = NamedSharding(mesh, PartitionSpec("core"))
        n_io = len(in_names) + len(out_names)
        self.fn = jax.jit(
            shard_map(_body, mesh=mesh,
                      in_specs=(PartitionSpec("core"),) * n_io,
                      out_specs=(PartitionSpec("core"),) * len(out_names),
                      check_rep=False),
            keep_unused=True,
        )
        self.dev_zeros = [
            jax.device_put(np.zeros((n_cores * z.shape[0], *z.shape[1:]),
                                    z.dtype), self.sharding)
            for z in zero_outs
        ]
        self.resident = {}

    def put_resident(self, name, per_core_arrays):
        cat = np.concatenate([np.asarray(a) for a in per_core_arrays], axis=0)
        self.resident[name] = self.jax.device_put(cat, self.sharding)

    def dispatch(self, **per_call):
        """Async: returns un-fetched global output arrays."""
        args = []
        for name in self.in_names:
            if name in per_call:
                v = per_call[name]
                if isinstance(v, (list, tuple)):
                    v = np.concatenate([np.asarray(a) for a in v], axis=0)
                args.append(self.jax.device_put(v, self.sharding))
            else:
                args.append(self.resident[name])
        return self.fn(*args, *self.dev_zeros)

    def fetch(self, outs):
        host = [np.asarray(o) for o in outs]
        return [
            {name: host[i].reshape(self.n_cores, *self.out_avals[i].shape)[c]
             for i, name in enumerate(self.out_names)}
            for c in range(self.n_cores)
        ]

    def __call__(self, **per_call):
        return self.fetch(self.dispatch(**per_call))


# ---------------- host-side prep ----------------

GPERM = np.concatenate([
    np.arange(0, 512), np.arange(512, 1024),
    np.arange(1536, 2048), np.arange(1024, 1536)])  # i,f,o,g tile order


def _prep_weights(w_ih_f, w_hh_f, b_f, w_ih_b, w_hh_b, b_b,
                  w_enc, b_enc, w_out, b_out, trans, embed):
    bf = ml_dtypes.bfloat16
    whh_t = np.empty((P, 2 * NG * NK, P), np.float32)
    wih_t = np.empty((P, 2 * NG * NE, P), np.float32)
    bias = np.empty((P, 2, NG), np.float32)
    for d, (wih_d, whh_d, b_d) in enumerate(
            [(w_ih_f, w_hh_f, b_f), (w_ih_b, w_hh_b, b_b)]):
        Whh = whh_d[GPERM]
        Wih = wih_d[GPERM]
        for m in range(NG):
            for k in range(NK):
                whh_t[:, (d * NG + m) * NK + k, :] = \
                    Whh[m * P:(m + 1) * P, k * P:(k + 1) * P].T
            for e in range(NE):
                wih_t[:, (d * NG + m) * NE + e, :] = \
                    Wih[m * P:(m + 1) * P, e * P:(e + 1) * P].T
        bias[:, d, :] = b_d[GPERM].reshape(NG, P).T

    wenc_t = np.empty((P, 2 * NK * NK, P), np.float32)
    for dd in range(2):
        Wd = w_enc[:, dd * H:(dd + 1) * H]
        for m in range(NK):
            for k in range(NK):
                wenc_t[:, dd * 16 + m * NK + k, :] = \
                    Wd[m * P:(m + 1) * P, k * P:(k + 1) * P].T
    benc_t = b_enc.reshape(NK, P).T.copy()
    wout_t = np.empty((P, NK, K), np.float32)
    for k in range(NK):
        wout_t[:, k, :] = w_out[:, k * P:(k + 1) * P].T

    pp = np.zeros((K, K + 1), np.float32)
    pp[:, :K] = np.exp(trans.astype(np.float64)).T.astype(np.float32)
    pp[:, K] = 1.0
    a0 = np.zeros((K, CB), np.float32)
    a0[0, :] = 1.0

    ident = np.eye(P, dtype=np.float32)
    rev = np.zeros((P, P), np.float32)
    tt, bb_ = np.meshgrid(np.arange(P // BS), np.arange(BS), indexing='ij')
    rev[(P // BS - 1 - tt) * BS + bb_, tt * BS + bb_] = 1.0
    perm = np.stack([ident, rev], axis=1)

    return {
        "emb": embed.astype(bf),
        "perm": perm.astype(bf),
        "whh": whh_t.astype(bf), "wih": wih_t.astype(bf),
        "bias": bias,
        "wenc": wenc_t.astype(bf), "benc": benc_t,
        "wout": wout_t.astype(bf), "bout": b_out.reshape(K, 1).astype(np.float32),
        "pp": pp.astype(bf), "a0": a0.astype(bf),
    }


def _weights_sig(arrs):
    # The dispatch->fetch window does no host work on the wire (commands are
    # flushed at the blocking fetch), so this runs on the critical path —
    # keep it ~1ms. ~16K strided samples per large array still detect any
    # realistic weight change (a regenerated array differs everywhere).
    h = hashlib.blake2b(digest_size=16)
    for a in arrs:
        a = np.asarray(a)
        h.update(str(a.shape).encode())
        if a.nbytes <= 1 << 16:
            h.update(np.ascontiguousarray(a).tobytes())
        else:
            flat = a.reshape(-1)
            stride = max(1, flat.size // 16384)
            h.update(np.ascontiguousarray(flat[::stride]).tobytes())
            h.update(np.ascontiguousarray(flat[-64:]).tobytes())
    return h.digest()


def _put_weights(runner, wlist, sig):
    prepped = _prep_weights(*[np.asarray(w, dtype=np.float32)
                              for w in wlist[1:]],
                            np.asarray(wlist[0], dtype=np.float32))
    for name, arr in prepped.items():
        runner.put_resident(name, [arr] * 8)
    _CACHE["sig"] = sig
    _CACHE["etstop"] = np.exp(np.asarray(wlist[-1])[K - 1].astype(np.float64))


def kernel(tokens, embed, w_ih_f, w_hh_f, b_f, w_ih_b, w_hh_b, b_b,
           w_enc, b_enc, w_out, b_out, trans):
    tokens = np.asarray(tokens)
    if tokens.dtype != np.int32:
        tokens = tokens.astype(np.int32)
    wlist = [embed, w_ih_f, w_hh_f, b_f, w_ih_b, w_hh_b, b_b,
             w_enc, b_enc, w_out, b_out, trans]

    if "runner" not in _CACHE:
        _CACHE["runner"] = _Runner(build(), 8)
        _put_weights(_CACHE["runner"], wlist, _weights_sig(wlist))
    runner = _CACHE["runner"]

    toks = [np.ascontiguousarray(tokens[:, c * BS:(c + 1) * BS])
            .reshape(NGT, P).astype(np.uint16) for c in range(8)]
    # dispatch optimistically with the resident weights, then verify the
    # weight signature while the device runs; redo on mismatch (rare).
    # Retry on transient proxy/transport errors.
    last_err = None
    for attempt in range(3):
        try:
            outs = runner.dispatch(tok=toks)
            sig = _weights_sig(wlist)
            if _CACHE["sig"] != sig:
                _put_weights(runner, wlist, sig)
                outs = runner.dispatch(tok=toks)
            res = runner.fetch(outs)
            break
        except Exception as e:  # noqa: BLE001 - re-raised after retries
            last_err = e
            import time as _time
            _time.sleep(0.5 * (attempt + 1))
    else:
        raise last_err

    etstop = _CACHE["etstop"]
    nzb = CB * (T // ZR)
    out = np.empty((B,), np.float32)
    for c in range(8):
        flat = np.asarray(res[c]["res"], dtype=np.float64).reshape(-1)
        zb = flat[:nzb].reshape(CB, T // ZR)
        af = flat[nzb:].reshape(K, CB)
        lz = np.log(zb).sum(axis=1) + np.log(etstop @ af)
        out[c * BS:(c + 1) * BS] = lz.astype(np.float32)
    return out
